# revision 5
# baseline (speedup 1.0000x reference)
"""Single-head memory attention on Trainium2, batch-parallel across 8 NeuronCores.

Per core (one batch element):
    Q^T = Wq @ x^T + bq                  (MM1, bf16, fp32 accum)
    S^T = keys @ Q^T                     (MM2; k on partitions, q on free dim)
    E^T = exp(S^T/sqrt(d) + mask_k)      (one ScalarE activation: scale+bias+exp)
    den = ones^T @ (sum_kt E^T)          (DVE tree-sum over k-tiles, then 4
                                          N=1 matmuls -> den lands [q,1] in PSUM)
    O   = E^T.T @ V  * recip(den)        (MM3 + per-partition normalize)

Operand transposes (x^T, keys^T, Wq^T) are 128x128 TensorE transposes.
(The DMA-xbar transpose path is faster on paper but Tile must globally
serialize all DMA around every transpose-mode transfer — a known HW-hang
workaround — which starves the whole pipeline.)

Scheduling notes:
- A burst of dummy transposes at t=0 keeps the PE HAM activity monitor
  busy so the clock gate opens (1.2 -> 2.4 GHz) before real work arrives;
  otherwise everything until ~40us runs at half clock.
- x chunk 0 is staged before Wq, and Wq staging is emitted interleaved
  with MM1 accumulation groups (MM1 group et only needs Wq row-block et),
  so the PE never sits behind the full 4MB Wq DMA.
- Chunk 0's MM3 runs kt-outer across concurrently-open PSUM groups so it
  consumes V row-blocks as they stream in; later chunks have V resident
  and run group-at-a-time.
- Rows whose additive mask is <= -1e8 contribute exactly 0 after exp, so
  the host gathers valid key rows and runs a smaller LK when possible.
"""

import numpy as np

import concourse.bacc as bacc
import concourse.mybir as mybir
from concourse.tile import TileContext
from concourse.masks import make_identity
from concourse.bass_utils import run_bass_kernel_spmd

B, LQ, D = 8, 2048, 1024
LK_FULL = 2048
P = 128
QCH = 512                 # queries processed per chunk
NQC = LQ // QCH           # 4 chunks
NDT = D // P              # 8 tiles along d (contraction of MM1)
NET = D // P              # 8 tiles along e (contraction of MM2)
NQS = QCH // P            # 4 query subtiles per chunk
SCALE = 1.0 / float(np.sqrt(D))
WARMUP = 48               # dummy transposes to open the HAM clock gate
NEG_INF = -1.0e9

F32 = mybir.dt.float32
BF16 = mybir.dt.bfloat16
AFT = mybir.ActivationFunctionType
AX = mybir.AxisListType
ALU = mybir.AluOpType

_CACHE = {}


def build_nc(nkt):
    lk = nkt * P
    nc = bacc.Bacc(None, target_bir_lowering=False)

    x_d = nc.dram_tensor("x", [LQ, D], F32, kind="ExternalInput")
    keys_d = nc.dram_tensor("keys", [lk, D], F32, kind="ExternalInput")
    values_d = nc.dram_tensor("values", [lk, D], F32, kind="ExternalInput")
    mask_d = nc.dram_tensor("mask", [lk, 1], F32, kind="ExternalInput")
    wq_d = nc.dram_tensor("Wq", [D, D], F32, kind="ExternalInput")
    bq_d = nc.dram_tensor("bq", [D], F32, kind="ExternalInput")
    out_d = nc.dram_tensor("out", [LQ, D], F32, kind="ExternalOutput")

    with TileContext(nc) as tc:
        with (
            tc.tile_pool(name="persist", bufs=1) as persist,
            tc.tile_pool(name="stage", bufs=8) as stagep,
            tc.tile_pool(name="cvt", bufs=4) as cvtp,
            tc.tile_pool(name="xTp", bufs=2) as xTp,
            tc.tile_pool(name="QTp", bufs=2) as QTp,
            tc.tile_pool(name="ETp", bufs=2) as ETp,
            tc.tile_pool(name="osb", bufs=5) as osbp,
            tc.tile_pool(name="esum", bufs=2) as esump,
            tc.tile_pool(name="rcp", bufs=2) as rcp,
            tc.tile_pool(name="psT", bufs=2, space="PSUM") as psTp,
            tc.tile_pool(name="psAcc", bufs=5, space="PSUM") as psAccp,
        ):
            # ---- constants ----
            ident = persist.tile([P, P], BF16)
            make_identity(nc, ident)
            ones_f32 = persist.tile([P, 1], F32)
            nc.any.memset(ones_f32, 1.0)
            bq_sb = persist.tile([P, NDT], F32)
            mask_sb = persist.tile([P, nkt], F32)

            # ---- persistent operands ----
            WqT = persist.tile([P, NDT, D], BF16)    # [d%P, d//P, e] = Wq[e, d]
            keysT = persist.tile([P, NET, lk], BF16)  # [e%P, e//P, k] = keys[k, e]
            Vsb = persist.tile([P, nkt, D], BF16)    # [k%P, k//P, dv] = values[k, dv]

            copy_eng = [
                lambda o, i: nc.vector.tensor_copy(o, i),
                lambda o, i: nc.scalar.copy(o, i),
            ]
            state = {"n": 0}

            # PE warm-up: HAM opens the clock gate after ~3.4us of sustained
            # PE activity; these run while the first DMAs stream in.
            warm_pt = psTp.tile([P, NDT, P], BF16, tag="pst")
            for _ in range(WARMUP):
                nc.tensor.transpose(warm_pt[:, 0, :], ident, ident)

            def transpose_block(dst3, col0, cv):
                # dst3[:, ft, col0:col0+P] = cv[:, ft*P:(ft+1)*P].T for ft in 0..7
                # All eight 128x128 transposes of one staged tile fill exactly
                # one 2KB PSUM bank, drained by a single strided copy.
                pt = psTp.tile([P, NDT, P], BF16, tag="pst")
                for ft in range(NDT):
                    nc.tensor.transpose(
                        pt[:, ft, :], cv[:, ft * P:(ft + 1) * P], ident
                    )
                copy_eng[state["n"] % 2](dst3[:, :, col0:col0 + P], pt)
                state["n"] += 1

            def stage_rows(dram_rows, parity):
                st = stagep.tile([P, D], F32, tag="stage")
                nc.sync.dma_start(st, dram_rows)
                cv = cvtp.tile([P, D], BF16, tag="cvt")
                cvt = nc.vector.tensor_copy if parity % 2 == 0 else nc.scalar.copy
                cvt(cv, st)
                return cv

            # x chunk staging: xT[p, dt, q'] = x[qc*QCH+q', dt*P+p]
            def x_stage(qc):
                xT = xTp.tile([P, NDT, QCH], BF16, tag="xT")
                for qs in range(NQS):
                    r0 = qc * QCH + qs * P
                    cv = stage_rows(x_d[r0:r0 + P, :], qs)
                    transpose_block(xT, qs * P, cv)
                return xT

            def mm1_group(xT, QT, et):
                # QT[e, q] = Wq @ x^T + bq, one 128-row block of e
                pq = psAccp.tile([P, QCH], F32, tag="acc")
                for dt in range(NDT):
                    nc.tensor.matmul(
                        pq,
                        WqT[:, dt, et * P:(et + 1) * P],
                        xT[:, dt, :],
                        start=(dt == 0),
                        stop=(dt == NDT - 1),
                    )
                nc.vector.tensor_scalar_add(QT[:, et, :], pq, bq_sb[:, et:et + 1])

            def mm2_group(QT, ET, kt):
                # S^T k-block + exp (scale+mask-bias fused into the activation)
                ps = psAccp.tile([P, QCH], F32, tag="acc")
                for et in range(NET):
                    nc.tensor.matmul(
                        ps,
                        keysT[:, et, kt * P:(kt + 1) * P],
                        QT[:, et, :],
                        start=(et == 0),
                        stop=(et == NET - 1),
                    )
                nc.scalar.activation(
                    ET[:, kt, :], ps, AFT.Exp,
                    bias=mask_sb[:, kt:kt + 1], scale=SCALE,
                )

            def reduce_den(ET):
                # sum E^T over the kt tile axis: [P, nkt, QCH] -> [P, QCH]
                esum = esump.tile([P, QCH], F32, tag="esum")
                nc.vector.tensor_reduce(
                    esum, ET.transpose([0, 2, 1]), axis=AX.X, op=ALU.add
                )
                return esum

            def den_recip(esum):
                # den[q, qs] = sum_p esum[p, qs*P+q]: q lands on partitions,
                # exactly the layout the per-partition normalize scale needs.
                den = psAccp.tile([P, NQS], F32, tag="den", bufs=1)
                for qs in range(NQS):
                    nc.tensor.matmul(
                        den[:, qs:qs + 1],
                        esum[:, qs * P:(qs + 1) * P],
                        ones_f32,
                    )
                rc = rcp.tile([P, NQS], F32, tag="rc")
                nc.vector.reciprocal(rc, den)
                return rc

            def mm3_norm(qc, po, rc, qs, dv, gi):
                # normalize + store one [128, 512] output block
                # (drains alternate DVE / ACT so neither engine's queue
                # becomes the po-recycling bottleneck)
                osb = osbp.tile([P, QCH], F32, tag="osb")
                if gi % 2 == 0:
                    nc.vector.tensor_scalar_mul(osb, po, rc[:, qs:qs + 1])
                else:
                    nc.scalar.activation(
                        osb, po, AFT.Copy, bias=0.0, scale=rc[:, qs:qs + 1],
                    )
                nc.sync.dma_start(
                    out_d[qc * QCH + qs * P: qc * QCH + (qs + 1) * P,
                          dv * QCH:(dv + 1) * QCH],
                    osb,
                )

            GROUPS = [(qs, dv) for qs in range(NQS) for dv in range(2)]

            def mm3_mm(po, ET, qs, dv, kt):
                nc.tensor.matmul(
                    po,
                    ET[:, kt, qs * P:(qs + 1) * P],
                    Vsb[:, kt, dv * QCH:(dv + 1) * QCH],
                    start=(kt == 0),
                    stop=(kt == nkt - 1),
                )

            def mm3_chunk0(ET):
                # V row-blocks are still streaming in: run kt-outer over
                # concurrently-open PSUM groups (5 then 3 — the acc ring is 5)
                # so each V block is consumed as it lands.
                esum = reduce_den(ET)
                rc = None
                for lo, hi in ((0, 5), (5, 8)):
                    pos = [psAccp.tile([P, QCH], F32, tag="acc", name=f"po{g}")
                           for g in range(lo, hi)]
                    for kt in range(nkt):
                        for po, (qs, dv) in zip(pos, GROUPS[lo:hi]):
                            mm3_mm(po, ET, qs, dv, kt)
                    if rc is None:
                        rc = den_recip(esum)
                    for po, (qs, dv) in zip(pos, GROUPS[lo:hi]):
                        mm3_norm(0, po, rc, qs, dv, qs * 2 + dv)

            def mm3_steady(qc, ET):
                esum = reduce_den(ET)
                rc = None
                pending = None
                for gi, (qs, dv) in enumerate(GROUPS):
                    po = psAccp.tile([P, QCH], F32, tag="acc")
                    for kt in range(nkt):
                        mm3_mm(po, ET, qs, dv, kt)
                    if gi == 0:
                        rc = den_recip(esum)
                    if pending is not None:
                        mm3_norm(qc, *pending)
                    pending = (po, rc, qs, dv, gi)
                mm3_norm(qc, *pending)

            # ---- emission ----
            xT0 = x_stage(0)
            nc.sync.dma_start(bq_sb, bq_d[:].rearrange("(t p) -> p t", p=P))
            nc.sync.dma_start(
                mask_sb, mask_d[:].rearrange("(t p) o -> p (t o)", p=P)
            )
            # Wq staged block-by-block, each immediately consumed by its MM1
            # accumulation group — the PE chases the Wq DMA instead of
            # waiting for it.
            QT0 = QTp.tile([P, NET, QCH], BF16, tag="QT")
            for et in range(NET):
                cv = stage_rows(wq_d[et * P:(et + 1) * P, :], et)
                transpose_block(WqT, et * P, cv)
                mm1_group(xT0, QT0, et)
            # keys likewise: transpose block kt feeds MM2 k-block kt.
            ET0 = ETp.tile([P, nkt, QCH], BF16, tag="ET")
            for kt in range(nkt):
                cv = stage_rows(keys_d[kt * P:(kt + 1) * P, :], kt)
                transpose_block(keysT, kt * P, cv)
                mm2_group(QT0, ET0, kt)
            # values -> Vsb (no transpose)
            for kt in range(nkt):
                st = stagep.tile([P, D], F32, tag="stage")
                nc.sync.dma_start(st, values_d[kt * P:(kt + 1) * P, :])
                cvt = nc.vector.tensor_copy if kt % 2 == 0 else nc.scalar.copy
                cvt(Vsb[:, kt, :], st)
            xT_next = x_stage(1)
            mm3_chunk0(ET0)
            for qc in range(1, NQC):
                QT = QTp.tile([P, NET, QCH], BF16, tag="QT")
                for et in range(NET):
                    mm1_group(xT_next, QT, et)
                ET = ETp.tile([P, nkt, QCH], BF16, tag="ET")
                for kt in range(nkt):
                    mm2_group(QT, ET, kt)
                if qc + 1 < NQC:
                    xT_next = x_stage(qc + 1)
                mm3_steady(qc, ET)

    nc.finalize()
    return nc


def _get_nc(nkt=15):
    key = f"nc{nkt}"
    if key not in _CACHE:
        _CACHE[key] = build_nc(nkt)
    return _CACHE[key]


def _prep_shard(keys_b, values_b, mask_b, lk):
    """Gather rows whose mask doesn't force exp() to zero; pad to lk rows."""
    if lk == LK_FULL:
        return (
            np.ascontiguousarray(keys_b, dtype=np.float32),
            np.ascontiguousarray(values_b, dtype=np.float32),
            np.ascontiguousarray(mask_b, dtype=np.float32),
        )
    keep = np.flatnonzero(mask_b[:, 0] > -1.0e8)
    n = len(keep)
    idx = np.zeros(lk, dtype=np.int64)
    idx[:n] = keep
    mask_g = np.full((lk, 1), NEG_INF, dtype=np.float32)
    mask_g[:n, 0] = mask_b[keep, 0]
    return (
        np.ascontiguousarray(keys_b[idx], dtype=np.float32),
        np.ascontiguousarray(values_b[idx], dtype=np.float32),
        mask_g,
    )


def kernel(x, mem_padding_mask, keys, values, Wq, bq):
    mask_np = np.asarray(mem_padding_mask, dtype=np.float32)
    n_valid = (mask_np[:, :, 0] > -1.0e8).sum(axis=1)
    nkt = 15 if n_valid.max() <= 15 * P else LK_FULL // P
    lk = nkt * P

    nc = _get_nc(nkt)
    Wq_c = np.ascontiguousarray(Wq, dtype=np.float32)
    bq_c = np.ascontiguousarray(bq, dtype=np.float32)
    in_maps = []
    for b in range(B):
        k_g, v_g, m_g = _prep_shard(
            np.asarray(keys[b]), np.asarray(values[b]), mask_np[b], lk
        )
        in_maps.append({
            "x": np.ascontiguousarray(x[b], dtype=np.float32),
            "keys": k_g,
            "values": v_g,
            "mask": m_g,
            "Wq": Wq_c,
            "bq": bq_c,
        })
    res = run_bass_kernel_spmd(nc, in_maps, core_ids=list(range(B)))
    return np.stack(
        [res.results[i]["out"] for i in range(B)], axis=0
    ).astype(np.float32)


# revision 6
# speedup vs baseline: 1.1001x; 1.1001x over previous
"""Single-head memory attention on Trainium2, batch-parallel across 8 NeuronCores.

Per core (one batch element):
    Q^T = Wq @ x^T + bq                  (MM1, bf16, fp32 accum)
    S^T = keys @ Q^T                     (MM2; k on partitions, q on free dim)
    E^T = exp(S^T/sqrt(d) + mask_k)      (one ScalarE activation: scale+bias+exp)
    den = ones^T @ (sum_kt E^T)          (DVE running-sum over k-tiles, then 8
                                          tiny fp16 matmuls -> den lands [q,1]
                                          in PSUM, in normalize layout)
    O   = E^T.T @ V  * recip(den)        (MM3 + per-partition normalize)

Operand transposes (x^T, keys^T, Wq^T) are 128x128 TensorE transposes.
(The DMA-xbar transpose path is faster on paper but Tile must globally
serialize all DMA around every transpose-mode transfer — a known HW-hang
workaround — which starves the whole pipeline.)

Scheduling notes:
- A short burst of dummy transposes at t=0 plus the x0 transposes keeps the
  PE HAM activity monitor busy so the clock gate opens (1.2 -> 2.4 GHz)
  before MM1; DMA-paced gaps otherwise keep re-throttling the PE.
- The staging ring is 4 deep on purpose: it throttles in-flight staging
  DMAs so x0 gets the HBM bandwidth first, then Wq, then keys, then V —
  all queued DMAs otherwise share bandwidth round-robin and x0 lands last.
- Wq/keys staging is emitted interleaved with their consumer matmul
  groups (MM1 group et only needs Wq row-block et) so the PE chases the
  DMA stream instead of waiting for it.
- The denominator partial sums run on the DVE interleaved with MM2 (one
  add per exp'd k-tile); a monolithic strided tensor_reduce measures
  ~13us and blocks the Vector FIFO (and with it the next chunk's staging).
- Chunk 0's MM3 runs kt-outer across 5+3 concurrently-open PSUM groups so
  it consumes V row-blocks as they stream in; later chunks have V
  resident and run group-at-a-time.
- Rows whose additive mask is <= -1e8 contribute exactly 0 after exp, so
  the host gathers valid key rows and runs a smaller LK when possible.
"""

import numpy as np

import concourse.bacc as bacc
import concourse.mybir as mybir
from concourse.tile import TileContext
from concourse.masks import make_identity
from concourse.bass_utils import run_bass_kernel_spmd

B, LQ, D = 8, 2048, 1024
LK_FULL = 2048
P = 128
QCH = 512                 # queries processed per chunk
NQC = LQ // QCH           # 4 chunks
NDT = D // P              # 8 tiles along d (contraction of MM1)
NET = D // P              # 8 tiles along e (contraction of MM2)
NQS = QCH // P            # 4 query subtiles per chunk
SCALE = 1.0 / float(np.sqrt(D))
WARMUP = 24               # dummy transposes to start opening the HAM clock gate
NEG_INF = -1.0e9

F32 = mybir.dt.float32
FP16 = mybir.dt.float16
BF16 = mybir.dt.bfloat16
AFT = mybir.ActivationFunctionType
ALU = mybir.AluOpType

_CACHE = {}


def build_nc(nkt):
    lk = nkt * P
    kt_b0 = nkt // 2 + 1      # first k-tile of the second denominator half

    nc = bacc.Bacc(None, target_bir_lowering=False)

    x_d = nc.dram_tensor("x", [LQ, D], F32, kind="ExternalInput")
    keys_d = nc.dram_tensor("keys", [lk, D], F32, kind="ExternalInput")
    values_d = nc.dram_tensor("values", [lk, D], F32, kind="ExternalInput")
    mask_d = nc.dram_tensor("mask", [lk, 1], F32, kind="ExternalInput")
    wq_d = nc.dram_tensor("Wq", [D, D], F32, kind="ExternalInput")
    bq_d = nc.dram_tensor("bq", [D], F32, kind="ExternalInput")
    out_d = nc.dram_tensor("out", [LQ, D], F32, kind="ExternalOutput")

    with TileContext(nc) as tc:
        with (
            tc.tile_pool(name="persist", bufs=1) as persist,
            tc.tile_pool(name="stage", bufs=4) as stagep,
            tc.tile_pool(name="cvt", bufs=4) as cvtp,
            tc.tile_pool(name="xTp", bufs=2) as xTp,
            tc.tile_pool(name="QTp", bufs=2) as QTp,
            tc.tile_pool(name="ETp", bufs=2) as ETp,
            tc.tile_pool(name="osb", bufs=5) as osbp,
            tc.tile_pool(name="esum", bufs=2) as esump,
            tc.tile_pool(name="rcp", bufs=2) as rcp,
            tc.tile_pool(name="psT", bufs=2, space="PSUM") as psTp,
            tc.tile_pool(name="psAcc", bufs=5, space="PSUM") as psAccp,
        ):
            # ---- constants ----
            ident = persist.tile([P, P], BF16)
            make_identity(nc, ident)
            ones16 = persist.tile([P, 1], FP16)
            nc.any.memset(ones16, 1.0)
            bq_sb = persist.tile([P, NDT], F32)
            mask_sb = persist.tile([P, nkt], F32)

            # ---- persistent operands ----
            WqT = persist.tile([P, NDT, D], BF16)    # [d%P, d//P, e] = Wq[e, d]
            keysT = persist.tile([P, NET, lk], BF16)  # [e%P, e//P, k] = keys[k, e]
            Vsb = persist.tile([P, nkt, D], BF16)    # [k%P, k//P, dv] = values[k, dv]

            copy_eng = [
                lambda o, i: nc.vector.tensor_copy(o, i),
                lambda o, i: nc.scalar.copy(o, i),
            ]
            state = {"n": 0}

            # PE warm-up: with the x0 transposes right behind, the HAM sees
            # sustained activity and opens the clock gate before MM1 starts.
            warm_pt = psTp.tile([P, NDT, P], BF16, tag="pst")
            for _ in range(WARMUP):
                nc.tensor.transpose(warm_pt[:, 0, :], ident, ident)

            def transpose_block(dst3, col0, cv):
                # dst3[:, ft, col0:col0+P] = cv[:, ft*P:(ft+1)*P].T for ft in 0..7
                # All eight 128x128 transposes of one staged tile fill exactly
                # one 2KB PSUM bank, drained by a single strided copy.
                pt = psTp.tile([P, NDT, P], BF16, tag="pst")
                for ft in range(NDT):
                    nc.tensor.transpose(
                        pt[:, ft, :], cv[:, ft * P:(ft + 1) * P], ident
                    )
                copy_eng[state["n"] % 2](dst3[:, :, col0:col0 + P], pt)
                state["n"] += 1

            def stage_rows(dram_rows, parity):
                st = stagep.tile([P, D], F32, tag="stage")
                nc.sync.dma_start(st, dram_rows)
                cv = cvtp.tile([P, D], BF16, tag="cvt")
                cvt = nc.vector.tensor_copy if parity % 2 == 0 else nc.scalar.copy
                cvt(cv, st)
                return cv

            # x chunk staging: xT[p, dt, q'] = x[qc*QCH+q', dt*P+p]
            def x_stage(qc):
                xT = xTp.tile([P, NDT, QCH], BF16, tag="xT")
                for qs in range(NQS):
                    r0 = qc * QCH + qs * P
                    cv = stage_rows(x_d[r0:r0 + P, :], qs)
                    transpose_block(xT, qs * P, cv)
                return xT

            def mm1_group(xT, QT, et):
                # QT[e, q] = Wq @ x^T + bq, one 128-row block of e
                pq = psAccp.tile([P, QCH], F32, tag="acc")
                for dt in range(NDT):
                    nc.tensor.matmul(
                        pq,
                        WqT[:, dt, et * P:(et + 1) * P],
                        xT[:, dt, :],
                        start=(dt == 0),
                        stop=(dt == NDT - 1),
                    )
                nc.vector.tensor_scalar_add(QT[:, et, :], pq, bq_sb[:, et:et + 1])

            def mm2_group(QT, ET, kt):
                # S^T k-block + exp (scale+mask-bias fused into the activation)
                ps = psAccp.tile([P, QCH], F32, tag="acc")
                for et in range(NET):
                    nc.tensor.matmul(
                        ps,
                        keysT[:, et, kt * P:(kt + 1) * P],
                        QT[:, et, :],
                        start=(et == 0),
                        stop=(et == NET - 1),
                    )
                nc.scalar.activation(
                    ET[:, kt, :], ps, AFT.Exp,
                    bias=mask_sb[:, kt:kt + 1], scale=SCALE,
                )

            def esum_step(ET, kt, halves):
                # Denominator partial sums ride along with MM2 on the DVE:
                # one contiguous add per freshly-exp'd k-tile.
                esA, esB = halves
                if kt == 1:
                    nc.vector.tensor_add(esA, ET[:, 0, :], ET[:, 1, :])
                elif 1 < kt <= kt_b0 - 1:
                    nc.vector.tensor_add(esA, esA, ET[:, kt, :])
                elif kt == kt_b0 + 1:
                    nc.vector.tensor_add(esB, ET[:, kt_b0, :], ET[:, kt, :])
                elif kt > kt_b0 + 1:
                    nc.vector.tensor_add(esB, esB, ET[:, kt, :])

            def esum_halves():
                esA = esump.tile([P, QCH], F32, tag="esA")
                esB = esump.tile([P, QCH], F32, tag="esB")
                return esA, esB

            def esum_fp16(halves):
                esA, esB = halves
                esA16 = esump.tile([P, QCH], FP16, tag="esA16")
                esB16 = esump.tile([P, QCH], FP16, tag="esB16")
                nc.scalar.copy(esA16, esA)
                nc.scalar.copy(esB16, esB)
                return esA16, esB16

            def den_recip(halves16):
                # den[q, qs] = sum_p (esA16+esB16)[p, qs*P+q]: q lands on
                # partitions, exactly the layout the normalize scale wants.
                den = psAccp.tile([P, NQS], F32, tag="den", bufs=1)
                for qs in range(NQS):
                    for hi, h in enumerate(halves16):
                        nc.tensor.matmul(
                            den[:, qs:qs + 1],
                            h[:, qs * P:(qs + 1) * P],
                            ones16,
                            start=(hi == 0),
                            stop=(hi == 1),
                        )
                rc = rcp.tile([P, NQS], F32, tag="rc")
                nc.vector.reciprocal(rc, den)
                return rc

            def mm3_norm(qc, po, rc, qs, dv, gi):
                # normalize + store one [128, 512] output block
                # (drains alternate DVE / ACT so neither engine's queue
                # becomes the po-recycling bottleneck)
                osb = osbp.tile([P, QCH], F32, tag="osb")
                if gi % 2 == 0:
                    nc.vector.tensor_scalar_mul(osb, po, rc[:, qs:qs + 1])
                else:
                    nc.scalar.activation(
                        osb, po, AFT.Copy, bias=0.0, scale=rc[:, qs:qs + 1],
                    )
                nc.sync.dma_start(
                    out_d[qc * QCH + qs * P: qc * QCH + (qs + 1) * P,
                          dv * QCH:(dv + 1) * QCH],
                    osb,
                )

            GROUPS = [(qs, dv) for qs in range(NQS) for dv in range(2)]

            def mm3_mm(po, ET, qs, dv, kt):
                nc.tensor.matmul(
                    po,
                    ET[:, kt, qs * P:(qs + 1) * P],
                    Vsb[:, kt, dv * QCH:(dv + 1) * QCH],
                    start=(kt == 0),
                    stop=(kt == nkt - 1),
                )

            def mm3_chunk0(ET, halves):
                h16 = esum_fp16(halves)
                rc = None
                for lo, hi in ((0, 5), (5, 8)):
                    pos = [psAccp.tile([P, QCH], F32, tag="acc", name=f"po{g}")
                           for g in range(lo, hi)]
                    for kt in range(nkt):
                        for po, (qs, dv) in zip(pos, GROUPS[lo:hi]):
                            mm3_mm(po, ET, qs, dv, kt)
                    if rc is None:
                        rc = den_recip(h16)
                    for po, (qs, dv) in zip(pos, GROUPS[lo:hi]):
                        mm3_norm(0, po, rc, qs, dv, qs * 2 + dv)

            def mm3_steady(qc, ET, halves):
                h16 = esum_fp16(halves)
                rc = None
                pending = []
                for gi, (qs, dv) in enumerate(GROUPS):
                    po = psAccp.tile([P, QCH], F32, tag="acc")
                    for kt in range(nkt):
                        mm3_mm(po, ET, qs, dv, kt)
                    pending.append((po, qs, dv, gi))
                    if gi == 1:
                        rc = den_recip(h16)
                    if rc is not None and pending:
                        po_, qs_, dv_, gi_ = pending.pop(0)
                        mm3_norm(qc, po_, rc, qs_, dv_, gi_)
                for po_, qs_, dv_, gi_ in pending:
                    mm3_norm(qc, po_, rc, qs_, dv_, gi_)

            # ---- emission ----
            xT0 = x_stage(0)
            nc.sync.dma_start(bq_sb, bq_d[:].rearrange("(t p) -> p t", p=P))
            nc.sync.dma_start(
                mask_sb, mask_d[:].rearrange("(t p) o -> p (t o)", p=P)
            )
            # Wq staged block-by-block, each immediately consumed by its MM1
            # accumulation group.
            QT0 = QTp.tile([P, NET, QCH], BF16, tag="QT")
            for et in range(NET):
                cv = stage_rows(wq_d[et * P:(et + 1) * P, :], et)
                transpose_block(WqT, et * P, cv)
                mm1_group(xT0, QT0, et)
            # keys likewise: transpose block kt feeds MM2 k-block kt.
            ET0 = ETp.tile([P, nkt, QCH], BF16, tag="ET")
            halves0 = esum_halves()
            for kt in range(nkt):
                cv = stage_rows(keys_d[kt * P:(kt + 1) * P, :], kt)
                transpose_block(keysT, kt * P, cv)
                mm2_group(QT0, ET0, kt)
                esum_step(ET0, kt, halves0)
            # values -> Vsb (no transpose)
            for kt in range(nkt):
                st = stagep.tile([P, D], F32, tag="stage")
                nc.sync.dma_start(st, values_d[kt * P:(kt + 1) * P, :])
                cvt = nc.vector.tensor_copy if kt % 2 == 0 else nc.scalar.copy
                cvt(Vsb[:, kt, :], st)
            xT_next = x_stage(1)
            mm3_chunk0(ET0, halves0)
            for qc in range(1, NQC):
                QT = QTp.tile([P, NET, QCH], BF16, tag="QT")
                for et in range(NET):
                    mm1_group(xT_next, QT, et)
                if qc + 1 < NQC:
                    xT_next = x_stage(qc + 1)
                ET = ETp.tile([P, nkt, QCH], BF16, tag="ET")
                halves = esum_halves()
                for kt in range(nkt):
                    mm2_group(QT, ET, kt)
                    esum_step(ET, kt, halves)
                mm3_steady(qc, ET, halves)

    nc.finalize()
    return nc


def _get_nc(nkt=15):
    key = f"nc{nkt}"
    if key not in _CACHE:
        _CACHE[key] = build_nc(nkt)
    return _CACHE[key]


def _prep_shard(keys_b, values_b, mask_b, lk):
    """Gather rows whose mask doesn't force exp() to zero; pad to lk rows."""
    if lk == LK_FULL:
        return (
            np.ascontiguousarray(keys_b, dtype=np.float32),
            np.ascontiguousarray(values_b, dtype=np.float32),
            np.ascontiguousarray(mask_b, dtype=np.float32),
        )
    keep = np.flatnonzero(mask_b[:, 0] > -1.0e8)
    n = len(keep)
    idx = np.zeros(lk, dtype=np.int64)
    idx[:n] = keep
    mask_g = np.full((lk, 1), NEG_INF, dtype=np.float32)
    mask_g[:n, 0] = mask_b[keep, 0]
    return (
        np.ascontiguousarray(keys_b[idx], dtype=np.float32),
        np.ascontiguousarray(values_b[idx], dtype=np.float32),
        mask_g,
    )


def kernel(x, mem_padding_mask, keys, values, Wq, bq):
    mask_np = np.asarray(mem_padding_mask, dtype=np.float32)
    n_valid = (mask_np[:, :, 0] > -1.0e8).sum(axis=1)
    nkt = 15 if n_valid.max() <= 15 * P else LK_FULL // P
    lk = nkt * P

    nc = _get_nc(nkt)
    Wq_c = np.ascontiguousarray(Wq, dtype=np.float32)
    bq_c = np.ascontiguousarray(bq, dtype=np.float32)
    in_maps = []
    for b in range(B):
        k_g, v_g, m_g = _prep_shard(
            np.asarray(keys[b]), np.asarray(values[b]), mask_np[b], lk
        )
        in_maps.append({
            "x": np.ascontiguousarray(x[b], dtype=np.float32),
            "keys": k_g,
            "values": v_g,
            "mask": m_g,
            "Wq": Wq_c,
            "bq": bq_c,
        })
    res = run_bass_kernel_spmd(nc, in_maps, core_ids=list(range(B)))
    return np.stack(
        [res.results[i]["out"] for i in range(B)], axis=0
    ).astype(np.float32)
